# revision 61
# baseline (speedup 1.0000x reference)
"""Trainium2 Bass kernel for causal multi-head attention (B=2, T=2048, C=1024, H=16, D=64).

Sharding (8 NeuronCores): data-parallel over the 2 batches x tensor-parallel over
4 head-groups (4 heads each). Core c handles batch c//4, heads 4*(c%4)..4*(c%4)+3.
Each core computes its 4 heads' QKV projections, causal attention, and a partial
output projection against its slice of Wo's columns; the host sums the 4 partials
per batch (the row-parallel unshard).

v2 design notes (all aimed at keeping the PE matmul stream gap-free, since any
PE idle both wastes cycles and resets the 3us DVFS ramp back to 1.2GHz):
  - DMA issue order is the consumption order (wq m0, x tile-by-tile, wk, wv,
    rest of x, wo) so the first projection group waits on ~0.4MB, not 2.5MB.
  - Attention runs i-outer / head-pair-inner; each 512-wide q-range's output
    projection is emitted right after its normalize with half-per-unit pacing
    and acts as PE filler inside the later q-ranges' attention.
  - Projection rounds are emitted interleaved with the attention i-loop so
    the softmax EXPs (the ACT engine long pole) start as early as possible.
  - The softmax normalize chain is DMA-free and gpsimd-free: l rows are PE-
    broadcast via K=1 selector matmuls into PSUM, 1/l comes from the DVE
    (reciprocal_approx_fast, ~18 bits), and one full-height multiply
    normalizes a unit. Staging SBUF->SBUF DMAs would queue ~7us behind bulk
    ring traffic, and partition-offset APs break gpsimd/custom-DVE library
    ops on HW (CoreSim does not model either failure).
  - Normalize multiplies are deferred one unit and Y-projection tiles a half
    iteration so neither gates an engine stream on a chain that may lag.
  - Projection PSUM rounds are half-outer/k-inner so each [128,512] group's
    eviction copy overlaps the next group's matmuls (no 2-buf pool stall).
  - Y is written bf16 (host sums partials in fp32; ~0.1% extra error).
  - PSUM budget is exactly 8 banks: scores 2x[128,2,512] + AV pair 2x[128,512]
    + projections 2x[128,512].

Device algorithm (per core), all matmuls bf16 with fp32 PSUM accumulation:
  QT = WqS @ x^T            [256, T]   (d-major; head pair per 128-row block)
  KT = WkS @ x^T            [256, T]
  V  = x @ WvS^T            [T, 4, 66] (t-major, head-strided, ones column at 64)
  per q-tile i (512 wide), head-pair hk, key-tile j (128 wide, j <= 4i+3):
    S^T both heads -> one [128, 2, 512] PSUM tile via two row-group-packed
      matmuls (K=64 each, tile_position (0,0) and (64,0))
    P^T = exp(S^T / 8)      (one ACT op per pair tile; unsafe softmax)
    P^T *= tri-mask         (diagonal blocks; gpsimd)
    O^T_h[:, i] += V_h[j]^T @ P^T_h   (M=65: ones column accumulates l)
  per (i, hk): rec = 1/l via DVE approx recip, gpsimd partition-broadcast,
    in-place scale of O^T; then Y[i-range] = O^T-as-lhsT @ WoS^T -> DRAM
"""

import numpy as np

B, T, C = 2, 2048, 1024
H, D = 16, 64
HPC = 4  # heads per core
N_CORES = 8
DH = HPC * D  # 256: per-core projection width

_compiled = None


def _patch_act_tables():
    """Make Exp and Ln resolve to the one table set that holds both, so the
    softmax exps never thrash ACT_TABLE_LOADs."""
    import functools

    import concourse.hw_specs as hw_specs
    import concourse.mybir as mybir
    from concourse import bacc

    if getattr(bacc, "_act_tables_patched", False):
        return
    orig = hw_specs.get_activation_tables

    @functools.cache
    def patched(arch):
        tabs = {k: set(v) for k, v in orig(arch).items()}
        E = mybir.ActivationFunctionType.Exp
        L = mybir.ActivationFunctionType.Ln
        keep = "natural_log_exp_and_others"
        if keep in tabs and E in tabs[keep] and L in tabs[keep]:
            for name, fns in tabs.items():
                if name != keep:
                    fns.discard(E)
                    fns.discard(L)
        return tabs

    bacc.get_activation_tables = patched
    bacc._act_tables_patched = True


def _build():
    import concourse.bass as bass
    import concourse.mybir as mybir
    from concourse import bacc
    from concourse.tile import TileContext

    _patch_act_tables()

    dt = mybir.dt
    BF = dt.bfloat16
    F32 = dt.float32
    ts = bass.ts
    Act = mybir.ActivationFunctionType

    P = 128
    NQ = T // 512   # 4 q-tiles of 512
    NK = T // 128   # 16 key-tiles of 128
    KC = C // 128   # 8 contraction subtiles for the projections

    nc = bacc.Bacc("TRN2", target_bir_lowering=False, debug=False)

    xT_d = nc.dram_tensor("xT", [C, T], BF, kind="ExternalInput")
    wq_d = nc.dram_tensor("wqT", [C, DH], BF, kind="ExternalInput")
    wk_d = nc.dram_tensor("wkT", [C, DH], BF, kind="ExternalInput")
    wv_d = nc.dram_tensor("wvT", [C, DH], BF, kind="ExternalInput")
    wo_d = nc.dram_tensor("woT", [DH, C], BF, kind="ExternalInput")
    y_d = nc.dram_tensor("y", [T, C], BF, kind="ExternalOutput")

    xT_r = xT_d[:].rearrange("(ko p) (nq t) -> p ko nq t", p=P, nq=NQ)
    wq_r = wq_d[:].rearrange("(ko p) m -> p ko m", p=P)
    wk_r = wk_d[:].rearrange("(ko p) m -> p ko m", p=P)
    wv_r = wv_d[:].rearrange("(ko p) m -> p ko m", p=P)
    wo_r = wo_d[:].rearrange("(ko p) n -> p ko n", p=P)

    with TileContext(nc) as tc:
        with (
            tc.tile_pool(name="persist", bufs=1) as persist,
            tc.tile_pool(name="ptiles", bufs=28) as ptiles,
            tc.tile_pool(name="ytiles", bufs=8) as ytiles,
            tc.tile_pool(name="ltmp", bufs=4) as ltmp,
            tc.tile_pool(name="rbpool", bufs=4) as rbpool,
            tc.tile_pool(name="psum_s", bufs=2, space="PSUM") as psum_s,
            tc.tile_pool(name="psum_o", bufs=2, space="PSUM") as psum_o,
            tc.tile_pool(name="psum_p", bufs=2, space="PSUM") as psum_p,
        ):
            # ---- persistent SBUF tensors -------------------------------
            x_sb = persist.tile([P, KC, T], BF, tag="x")          # x^T
            wq_sb = persist.tile([P, KC, DH], BF, tag="wq")
            wk_sb = persist.tile([P, KC, DH], BF, tag="wk")
            wv_sb = persist.tile([P, KC, DH], BF, tag="wv")
            wo_sb = persist.tile([P, DH // P, C], BF, tag="wo")
            qT_sb = persist.tile([P, DH // P, T], BF, tag="qT")
            kT_sb = persist.tile([P, DH // P, T], BF, tag="kT")
            v_sb = persist.tile([P, NK, HPC, 66], BF, tag="v")
            oT_sb = persist.tile([P, DH // P, T], BF, tag="oT")
            cmask = persist.tile([P, 2, P], BF, tag="cmask")
            sel_t = persist.tile([P, 2, P], BF, tag="sel_t")

            # ---- DMAs in consumption order -----------------------------
            # wq m-halves and per-(ko, 512-tile) x chunks: the first
            # projection group only waits on ~0.4MB instead of ~2.5MB
            nc.sync.dma_start(wq_sb[:, :, 0:P], wq_r[:, :, 0:P])
            for ko in range(KC):
                nc.sync.dma_start(x_sb[:, ko, ts(0, 512)], xT_r[:, ko, 0:1, :])
            nc.sync.dma_start(wq_sb[:, :, P:DH], wq_r[:, :, P:DH])
            nc.sync.dma_start(wk_sb[:], wk_r)
            nc.sync.dma_start(wv_sb[:], wv_r)
            for ko in range(KC):
                nc.sync.dma_start(x_sb[:, ko, ts(1, 512)], xT_r[:, ko, 1:2, :])
            for ko in range(KC):
                nc.sync.dma_start(x_sb[:, ko, ts(2, 512)], xT_r[:, ko, 2:3, :])
            for ko in range(KC):
                nc.sync.dma_start(x_sb[:, ko, ts(3, 512)], xT_r[:, ko, 3:4, :])
            nc.sync.dma_start(wo_sb[:], wo_r)

            # ---- constants: causal corner mask + V's ones column -------
            # diagonal [128,128] corner: keep 1.0 where col >= row, else 0.0
            nc.gpsimd.memset(v_sb[:, :, :, 64:66], 1.0)
            # selector rows for the l-broadcast matmuls: the two K=1
            # accumulating matmuls route half h's l to out rows 64h..64h+63
            nc.gpsimd.memset(sel_t[:], 0.0)
            nc.gpsimd.memset(sel_t[64:65, 0, 0:64], 1.0)
            nc.gpsimd.memset(sel_t[64:65, 1, 64:128], 1.0)
            # causal corner mask: keep 1.0 where col >= row, else 0.0
            nc.gpsimd.memset(cmask[:], 1.0)
            for half in range(2):
                nc.gpsimd.affine_select(
                    out=cmask[:, half, :],
                    in_=cmask[:, half, :],
                    compare_op=mybir.AluOpType.is_ge,
                    fill=0.0,
                    base=0,
                    pattern=[[1, P]],
                    channel_multiplier=-1,
                )

            # ---- QT / KT projections (half-outer: copy of group r
            #      overlaps matmuls of group r+1 within the 2-buf pool) ---
            def qk_round(np2, half):
                for w_sb, out_sb in ((wq_sb, qT_sb), (wk_sb, kT_sb)):
                    for m in range(DH // P):
                        ps = psum_p.tile([P, 512], F32, tag="pp")
                        for k in range(KC):
                            nc.tensor.matmul(
                                ps[:],
                                w_sb[:, k, ts(m, P)],
                                x_sb[:, k, ts(2 * np2 + half, 512)],
                                start=(k == 0),
                                stop=(k == KC - 1),
                            )
                        nc.vector.tensor_copy(
                            out_sb[:, m, ts(2 * np2 + half, 512)], ps[:]
                        )

            # ---- V projection: out[t-tile, head, d] --------------------
            def v_round(mt):
                ps_full = psum_p.tile([P, 512], F32, tag="pp")
                ps = ps_full[:, :DH]
                for k in range(KC):
                    nc.tensor.matmul(
                        ps[:],
                        x_sb[:, k, ts(mt, P)],
                        wv_sb[:, k, :],
                        start=(k == 0),
                        stop=(k == KC - 1),
                    )
                nc.vector.tensor_copy(
                    v_sb[:, mt, :, 0:64], ps.rearrange("p (h d) -> p h d", d=64)
                )

            # projection rounds are emitted interleaved with the attention
            # i-loop below (see the i==0/1 injections) so the softmax EXPs —
            # the saturated ACT engine's long pole — start ~12us earlier,
            # with the remaining projections as PE filler

            # ---- output projection for part of one 512-wide t-range ----
            # mid-stream evictions stay off ACT (it is the EXP-bound engine
            # during attention); in the tail the EXPs are done, so alternate
            # DVE/ACT there so neither engine paces the final groups
            def y_proj(mts, tail=False):
                for mt in mts:
                    for n in range(C // 512):
                        py = psum_p.tile([P, 512], F32, tag="pp")
                        for kc in range(DH // P):
                            nc.tensor.matmul(
                                py[:],
                                oT_sb[:, kc, ts(mt, P)],
                                wo_sb[:, kc, ts(n, 512)],
                                start=(kc == 0),
                                stop=(kc == DH // P - 1),
                            )
                        yt = ytiles.tile([P, 512], BF, tag="y")
                        if tail and n == 1:
                            nc.scalar.copy(yt[:], py[:])
                        else:
                            nc.vector.tensor_copy(yt[:], py[:])
                        nc.sync.dma_start(y_d[ts(mt, P), ts(n, 512)], yt[:])

            # ---- attention: i-outer, head-pair-inner -------------------
            # pending_mults: the in-place normalize multiplies are deferred
            # one (i, hk) unit so the DVE stream never head-of-line blocks
            # on the recip/broadcast chain; pending_yproj defers each
            # t-range's output projection a half iteration for the same
            # reason (its PE matmuls must not gate on a lagging chain).
            pending_mults = []
            pending_yproj = []

            def flush_half_yproj():
                if pending_yproj:
                    y_proj(pending_yproj[:2])
                    del pending_yproj[:2]

            for i in range(NQ):
                if i == 0:
                    # everything attention i=0,1 needs: qT/kT over t 0-1023
                    # and V tiles 0-7; V(0-3) sits between the two QK halves
                    # so the PE has tile-0-only work while x tile-1 lands
                    qk_round(0, 0)
                    for mt in range(NK // 4):
                        v_round(mt)
                    qk_round(0, 1)
                    for mt in range(NK // 4, NK // 2):
                        v_round(mt)
                elif i == 1:
                    # the rest of the projections, as filler from here on
                    for half in range(2):
                        qk_round(1, half)
                    for mt in range(NK // 2, NK):
                        v_round(mt)
                for hk in range(DH // P):
                    prev_mults = pending_mults
                    pending_mults = []
                    jmax = 4 * i + 3
                    pts = []
                    for j in range(jmax + 1):
                        # diagonal tiles with offset t: columns < 128t are
                        # fully causal-masked, so skip computing them
                        c0 = P * (j - 4 * i) if j >= 4 * i else 0
                        sp = psum_s.tile([P, 2, 512], F32, tag="s")
                        # row-group-packed pair: head 2*hk in PE rows 0-63,
                        # head 2*hk+1 in rows 64-127
                        for half in range(2):
                            hp = 64 * half
                            nc.tensor.matmul(
                                sp[:, half, c0:],
                                kT_sb[hp : hp + 64, hk, ts(j, P)],
                                qT_sb[hp : hp + 64, hk, 512 * i + c0 : 512 * (i + 1)],
                                start=True,
                                stop=True,
                                tile_position=(hp, 0),
                            )
                        pt = ptiles.tile([P, 2, 512], BF, tag="p")
                        if j >= 4 * i:
                            t = j - 4 * i
                            # cols < 128t are never computed nor read
                            nc.scalar.activation(
                                pt[:, :, P * t :], sp[:, :, P * t :],
                                Act.Exp, scale=0.125,
                            )
                            nc.gpsimd.tensor_mul(
                                pt[:, :, P * t : P * (t + 1)],
                                pt[:, :, P * t : P * (t + 1)],
                                cmask[:],
                            )
                        else:
                            nc.scalar.activation(pt[:], sp[:], Act.Exp, scale=0.125)
                        pts.append(pt)
                    op0 = psum_o.tile([P, 512], F32, tag="o")
                    op1 = psum_o.tile([P, 512], F32, tag="o")
                    ops = [op0, op1]
                    for j in range(jmax + 1):
                        # diagonal tile t only contributes to columns >= 128t
                        # (pt is zero below); j=0 is always the full-width
                        # start=True writer, so partial-width accumulates are
                        # safe for every element
                        c0 = P * (j - 4 * i) if j >= 4 * i else 0
                        for half in range(2):
                            h = 2 * hk + half
                            nc.tensor.matmul(
                                ops[half][0:65, c0:],
                                v_sb[:, j, h, 0:65],
                                pts[j][:, half, c0:],
                                start=(j == 0),
                                stop=(j == jmax),
                            )
                    # evict O^T (unnormalized) + l rows, then normalize.
                    # The l row is broadcast across partitions with a K=1 PE
                    # matmul (ones lhsT) — deliberately NO DMA and NO gpsimd
                    # in this chain: staging DMAs queue behind the bulk x/Y
                    # ring traffic (~7us each on HW), and partition-offset
                    # APs break the gpsimd/custom-DVE library ops on HW.
                    # reciprocal_approx_fast (~18 bits) is plenty for bf16.
                    lt = ltmp.tile([P, 2, 512], BF, tag="lt")
                    # l rows first (they gate the bcast->recip->mult chain):
                    # half h's l lands on partition 64+h so one K=2 matmul
                    # against the selector rows broadcasts both (rows 0-63
                    # <- l_h0, rows 64-127 <- l_h1), then one recip and one
                    # full-height multiply normalize the whole unit.
                    for half in range(2):
                        nc.vector.tensor_copy(
                            lt[64:65, half, :], ops[half][64:65, :]
                        )
                    if i == NQ - 1:
                        # tail units: ACT is about to run dry of EXPs, so
                        # split the evictions across engines for latency
                        nc.scalar.copy(
                            oT_sb[0:64, hk, ts(i, 512)], ops[0][0:64, :]
                        )
                    else:
                        nc.vector.tensor_copy(
                            oT_sb[0:64, hk, ts(i, 512)], ops[0][0:64, :]
                        )
                    nc.vector.tensor_copy(
                        oT_sb[64:128, hk, ts(i, 512)], ops[1][0:64, :]
                    )
                    # NOT the psum_p ring: FIFO slot rotation there would
                    # make later projection groups wait on this unit's recip
                    # chain, serializing the PE filler behind attention
                    lb = psum_o.tile([P, 512], F32, tag="o")
                    for half in range(2):
                        nc.tensor.matmul(
                            lb[:],
                            sel_t[64:65, half, :],
                            lt[64:65, half, :],
                            start=(half == 0),
                            stop=(half == 1),
                        )
                    rec = rbpool.tile([P, 512], F32, tag="rec")
                    nc.vector.reciprocal_approx_fast(rec[:], lb[:])
                    pending_mults.append(
                        (
                            oT_sb[:, hk, ts(i, 512)],
                            oT_sb[:, hk, ts(i, 512)],
                            rec[:],
                        )
                    )
                    # flush the previous unit's deferred normalizes now that
                    # their chains have had a full unit of slack
                    for args in prev_mults:
                        nc.vector.tensor_mul(*args)
                    if hk == 0 and i > 0:
                        # Y for t-range i-1 becomes eligible here (both head
                        # pairs normalized: prev_mults flushed above); its 4
                        # tiles are spread half-per-unit as steady PE filler
                        pending_yproj.extend(range(4 * (i - 1), 4 * i))
                    flush_half_yproj()

            # tail: flush the last deferred normalizes and final Y ranges
            for args in pending_mults:
                nc.vector.tensor_mul(*args)
            y_proj(pending_yproj, tail=True)
            y_proj(range(4 * (NQ - 1), 4 * NQ), tail=True)

    nc.compile()
    return nc


def _get_compiled():
    global _compiled
    if _compiled is None:
        _compiled = _build()
    return _compiled


def make_inputs(x, Wq, Wk, Wv, Wo):
    """Shard the full inputs into the 8 per-core input maps (host-side prep)."""
    import ml_dtypes

    bf16 = ml_dtypes.bfloat16
    x = np.asarray(x)
    in_maps = []
    for c in range(N_CORES):
        b, g = divmod(c, HPC)
        rows = slice(g * DH, (g + 1) * DH)
        in_maps.append(
            {
                "xT": np.ascontiguousarray(x[b].T).astype(bf16),
                "wqT": np.ascontiguousarray(np.asarray(Wq)[rows, :].T).astype(bf16),
                "wkT": np.ascontiguousarray(np.asarray(Wk)[rows, :].T).astype(bf16),
                "wvT": np.ascontiguousarray(np.asarray(Wv)[rows, :].T).astype(bf16),
                "woT": np.ascontiguousarray(np.asarray(Wo)[:, rows].T).astype(bf16),
            }
        )
    return in_maps


def assemble(results):
    """Sum the 4 tensor-parallel partials per batch into the full output."""
    y = np.zeros((B, T, C), dtype=np.float32)
    for c in range(N_CORES):
        b = c // HPC
        y[b] += np.asarray(results[c]["y"]).astype(np.float32)
    return y


def kernel(x, Wq, Wk, Wv, Wo):
    from concourse.bass_utils import run_bass_kernel_spmd

    nc = _get_compiled()
    in_maps = make_inputs(x, Wq, Wk, Wv, Wo)
    res = run_bass_kernel_spmd(nc, in_maps, list(range(N_CORES)))
    return assemble(res.results)


# revision 62
# speedup vs baseline: 1.0000x; 1.0000x over previous
"""Trainium2 Bass kernel for causal multi-head attention (B=2, T=2048, C=1024, H=16, D=64).

Sharding (8 NeuronCores): data-parallel over the 2 batches x tensor-parallel over
4 head-groups (4 heads each). Core c handles batch c//4, heads 4*(c%4)..4*(c%4)+3.
Each core computes its 4 heads' QKV projections, causal attention, and a partial
output projection against its slice of Wo's columns; the host sums the 4 partials
per batch (the row-parallel unshard).

v2 design notes (all aimed at keeping the PE matmul stream gap-free, since any
PE idle both wastes cycles and resets the 3us DVFS ramp back to 1.2GHz):
  - DMA issue order is the consumption order (wq m0, x tile-by-tile, wk, wv,
    rest of x, wo) so the first projection group waits on ~0.4MB, not 2.5MB.
  - Attention runs i-outer / head-pair-inner; each 512-wide q-range's output
    projection is emitted right after its normalize with half-per-unit pacing
    and acts as PE filler inside the later q-ranges' attention.
  - Projection rounds are emitted interleaved with the attention i-loop so
    the softmax EXPs (the ACT engine long pole) start as early as possible.
  - The softmax normalize chain is DMA-free and gpsimd-free: l rows are PE-
    broadcast via K=1 selector matmuls into PSUM, 1/l comes from the DVE
    (reciprocal_approx_fast, ~18 bits), and one full-height multiply
    normalizes a unit. Staging SBUF->SBUF DMAs would queue ~7us behind bulk
    ring traffic, and partition-offset APs break gpsimd/custom-DVE library
    ops on HW (CoreSim does not model either failure).
  - Normalize multiplies are deferred one unit and Y-projection tiles a half
    iteration so neither gates an engine stream on a chain that may lag.
  - Projection PSUM rounds are half-outer/k-inner so each [128,512] group's
    eviction copy overlaps the next group's matmuls (no 2-buf pool stall).
  - Y is written bf16 (host sums partials in fp32; ~0.1% extra error).
  - PSUM budget is exactly 8 banks: scores 2x[128,2,512] + AV pair 2x[128,512]
    + projections 2x[128,512].

Device algorithm (per core), all matmuls bf16 with fp32 PSUM accumulation:
  QT = WqS @ x^T            [256, T]   (d-major; head pair per 128-row block)
  KT = WkS @ x^T            [256, T]
  V  = x @ WvS^T            [T, 4, 66] (t-major, head-strided, ones column at 64)
  per q-tile i (512 wide), head-pair hk, key-tile j (128 wide, j <= 4i+3):
    S^T both heads -> one [128, 2, 512] PSUM tile via two row-group-packed
      matmuls (K=64 each, tile_position (0,0) and (64,0))
    P^T = exp(S^T / 8)      (one ACT op per pair tile; unsafe softmax)
    P^T *= tri-mask         (diagonal blocks; gpsimd)
    O^T_h[:, i] += V_h[j]^T @ P^T_h   (M=65: ones column accumulates l)
  per (i, hk): rec = 1/l via DVE approx recip, gpsimd partition-broadcast,
    in-place scale of O^T; then Y[i-range] = O^T-as-lhsT @ WoS^T -> DRAM
"""

import numpy as np

B, T, C = 2, 2048, 1024
H, D = 16, 64
HPC = 4  # heads per core
N_CORES = 8
DH = HPC * D  # 256: per-core projection width

_compiled = None


def _patch_act_tables():
    """Make Exp and Ln resolve to the one table set that holds both, so the
    softmax exps never thrash ACT_TABLE_LOADs."""
    import functools

    import concourse.hw_specs as hw_specs
    import concourse.mybir as mybir
    from concourse import bacc

    if getattr(bacc, "_act_tables_patched", False):
        return
    orig = hw_specs.get_activation_tables

    @functools.cache
    def patched(arch):
        tabs = {k: set(v) for k, v in orig(arch).items()}
        E = mybir.ActivationFunctionType.Exp
        L = mybir.ActivationFunctionType.Ln
        keep = "natural_log_exp_and_others"
        if keep in tabs and E in tabs[keep] and L in tabs[keep]:
            for name, fns in tabs.items():
                if name != keep:
                    fns.discard(E)
                    fns.discard(L)
        return tabs

    bacc.get_activation_tables = patched
    bacc._act_tables_patched = True


def _build():
    import concourse.bass as bass
    import concourse.mybir as mybir
    from concourse import bacc
    from concourse.tile import TileContext

    _patch_act_tables()

    dt = mybir.dt
    BF = dt.bfloat16
    F32 = dt.float32
    ts = bass.ts
    Act = mybir.ActivationFunctionType

    P = 128
    NQ = T // 512   # 4 q-tiles of 512
    NK = T // 128   # 16 key-tiles of 128
    KC = C // 128   # 8 contraction subtiles for the projections

    nc = bacc.Bacc("TRN2", target_bir_lowering=False, debug=False)

    xT_d = nc.dram_tensor("xT", [C, T], BF, kind="ExternalInput")
    wq_d = nc.dram_tensor("wqT", [C, DH], BF, kind="ExternalInput")
    wk_d = nc.dram_tensor("wkT", [C, DH], BF, kind="ExternalInput")
    wv_d = nc.dram_tensor("wvT", [C, DH], BF, kind="ExternalInput")
    wo_d = nc.dram_tensor("woT", [DH, C], BF, kind="ExternalInput")
    y_d = nc.dram_tensor("y", [T, C], BF, kind="ExternalOutput")

    xT_r = xT_d[:].rearrange("(ko p) (nq t) -> p ko nq t", p=P, nq=NQ)
    wq_r = wq_d[:].rearrange("(ko p) m -> p ko m", p=P)
    wk_r = wk_d[:].rearrange("(ko p) m -> p ko m", p=P)
    wv_r = wv_d[:].rearrange("(ko p) m -> p ko m", p=P)
    wo_r = wo_d[:].rearrange("(ko p) n -> p ko n", p=P)

    with TileContext(nc) as tc:
        with (
            tc.tile_pool(name="persist", bufs=1) as persist,
            tc.tile_pool(name="ptiles", bufs=28) as ptiles,
            tc.tile_pool(name="ytiles", bufs=8) as ytiles,
            tc.tile_pool(name="ltmp", bufs=4) as ltmp,
            tc.tile_pool(name="rbpool", bufs=4) as rbpool,
            tc.tile_pool(name="psum_s", bufs=2, space="PSUM") as psum_s,
            tc.tile_pool(name="psum_o", bufs=2, space="PSUM") as psum_o,
            tc.tile_pool(name="psum_p", bufs=2, space="PSUM") as psum_p,
        ):
            # ---- persistent SBUF tensors -------------------------------
            x_sb = persist.tile([P, KC, T], BF, tag="x")          # x^T
            wq_sb = persist.tile([P, KC, DH], BF, tag="wq")
            wk_sb = persist.tile([P, KC, DH], BF, tag="wk")
            wv_sb = persist.tile([P, KC, DH], BF, tag="wv")
            wo_sb = persist.tile([P, DH // P, C], BF, tag="wo")
            qT_sb = persist.tile([P, DH // P, T], BF, tag="qT")
            kT_sb = persist.tile([P, DH // P, T], BF, tag="kT")
            v_sb = persist.tile([P, NK, HPC, 66], BF, tag="v")
            oT_sb = persist.tile([P, DH // P, T], BF, tag="oT")
            cmask = persist.tile([P, 2, P], BF, tag="cmask")
            sel_t = persist.tile([P, 2, P], BF, tag="sel_t")

            # ---- DMAs in consumption order -----------------------------
            # wq m-halves and per-(ko, 512-tile) x chunks: the first
            # projection group only waits on ~0.4MB instead of ~2.5MB
            nc.sync.dma_start(wq_sb[:, :, 0:P], wq_r[:, :, 0:P])
            for ko in range(KC):
                nc.sync.dma_start(x_sb[:, ko, ts(0, 512)], xT_r[:, ko, 0:1, :])
            nc.sync.dma_start(wq_sb[:, :, P:DH], wq_r[:, :, P:DH])
            nc.sync.dma_start(wk_sb[:], wk_r)
            nc.sync.dma_start(wv_sb[:], wv_r)
            for ko in range(KC):
                nc.sync.dma_start(x_sb[:, ko, ts(1, 512)], xT_r[:, ko, 1:2, :])
            for ko in range(KC):
                nc.sync.dma_start(x_sb[:, ko, ts(2, 512)], xT_r[:, ko, 2:3, :])
            for ko in range(KC):
                nc.sync.dma_start(x_sb[:, ko, ts(3, 512)], xT_r[:, ko, 3:4, :])
            nc.sync.dma_start(wo_sb[:], wo_r)

            # ---- constants: causal corner mask + V's ones column -------
            # diagonal [128,128] corner: keep 1.0 where col >= row, else 0.0
            nc.gpsimd.memset(v_sb[:, :, :, 64:66], 1.0)
            # selector rows for the l-broadcast matmuls: the two K=1
            # accumulating matmuls route half h's l to out rows 64h..64h+63
            nc.gpsimd.memset(sel_t[:], 0.0)
            nc.gpsimd.memset(sel_t[64:65, 0, 0:64], 1.0)
            nc.gpsimd.memset(sel_t[64:65, 1, 64:128], 1.0)
            # causal corner mask: keep 1.0 where col >= row, else 0.0
            nc.gpsimd.memset(cmask[:], 1.0)
            for half in range(2):
                nc.gpsimd.affine_select(
                    out=cmask[:, half, :],
                    in_=cmask[:, half, :],
                    compare_op=mybir.AluOpType.is_ge,
                    fill=0.0,
                    base=0,
                    pattern=[[1, P]],
                    channel_multiplier=-1,
                )

            # ---- QT / KT projections (half-outer: copy of group r
            #      overlaps matmuls of group r+1 within the 2-buf pool) ---
            def qk_round(np2, half):
                for w_sb, out_sb in ((wq_sb, qT_sb), (wk_sb, kT_sb)):
                    for m in range(DH // P):
                        ps = psum_p.tile([P, 512], F32, tag="pp")
                        for k in range(KC):
                            nc.tensor.matmul(
                                ps[:],
                                w_sb[:, k, ts(m, P)],
                                x_sb[:, k, ts(2 * np2 + half, 512)],
                                start=(k == 0),
                                stop=(k == KC - 1),
                            )
                        nc.vector.tensor_copy(
                            out_sb[:, m, ts(2 * np2 + half, 512)], ps[:]
                        )

            # ---- V projection: out[t-tile, head, d] --------------------
            def v_round(mt):
                ps_full = psum_p.tile([P, 512], F32, tag="pp")
                ps = ps_full[:, :DH]
                for k in range(KC):
                    nc.tensor.matmul(
                        ps[:],
                        x_sb[:, k, ts(mt, P)],
                        wv_sb[:, k, :],
                        start=(k == 0),
                        stop=(k == KC - 1),
                    )
                nc.vector.tensor_copy(
                    v_sb[:, mt, :, 0:64], ps.rearrange("p (h d) -> p h d", d=64)
                )

            # projection rounds are emitted interleaved with the attention
            # i-loop below (see the i==0/1 injections) so the softmax EXPs —
            # the saturated ACT engine's long pole — start ~12us earlier,
            # with the remaining projections as PE filler

            # ---- output projection for part of one 512-wide t-range ----
            # mid-stream evictions stay off ACT (it is the EXP-bound engine
            # during attention); in the tail the EXPs are done, so alternate
            # DVE/ACT there so neither engine paces the final groups
            def y_proj(mts, tail=False):
                for mt in mts:
                    for n in range(C // 512):
                        py = psum_p.tile([P, 512], F32, tag="pp")
                        for kc in range(DH // P):
                            nc.tensor.matmul(
                                py[:],
                                oT_sb[:, kc, ts(mt, P)],
                                wo_sb[:, kc, ts(n, 512)],
                                start=(kc == 0),
                                stop=(kc == DH // P - 1),
                            )
                        yt = ytiles.tile([P, 512], BF, tag="y")
                        if tail and n == 1:
                            nc.scalar.copy(yt[:], py[:])
                        else:
                            nc.vector.tensor_copy(yt[:], py[:])
                        nc.sync.dma_start(y_d[ts(mt, P), ts(n, 512)], yt[:])

            # ---- attention: i-outer, head-pair-inner -------------------
            # pending_mults: the in-place normalize multiplies are deferred
            # one (i, hk) unit so the DVE stream never head-of-line blocks
            # on the recip/broadcast chain; pending_yproj defers each
            # t-range's output projection a half iteration for the same
            # reason (its PE matmuls must not gate on a lagging chain).
            pending_mults = []
            pending_yproj = []

            def flush_half_yproj():
                if pending_yproj:
                    y_proj(pending_yproj[:2])
                    del pending_yproj[:2]

            for i in range(NQ):
                if i == 0:
                    # everything attention i=0,1 needs: qT/kT over t 0-1023
                    # and V tiles 0-7; V(0-3) sits between the two QK halves
                    # so the PE has tile-0-only work while x tile-1 lands
                    qk_round(0, 0)
                    for mt in range(NK // 4):
                        v_round(mt)
                    qk_round(0, 1)
                    for mt in range(NK // 4, NK // 2):
                        v_round(mt)
                elif i == 1:
                    # the rest of the projections, as filler from here on
                    for half in range(2):
                        qk_round(1, half)
                    for mt in range(NK // 2, NK):
                        v_round(mt)
                for hk in range(DH // P):
                    prev_mults = pending_mults
                    pending_mults = []
                    jmax = 4 * i + 3
                    pts = []
                    for j in range(jmax + 1):
                        # diagonal tiles with offset t: columns < 128t are
                        # fully causal-masked, so skip computing them
                        c0 = P * (j - 4 * i) if j >= 4 * i else 0
                        sp = psum_s.tile([P, 2, 512], F32, tag="s")
                        # row-group-packed pair: head 2*hk in PE rows 0-63,
                        # head 2*hk+1 in rows 64-127
                        for half in range(2):
                            hp = 64 * half
                            nc.tensor.matmul(
                                sp[:, half, c0:],
                                kT_sb[hp : hp + 64, hk, ts(j, P)],
                                qT_sb[hp : hp + 64, hk, 512 * i + c0 : 512 * (i + 1)],
                                start=True,
                                stop=True,
                                tile_position=(hp, 0),
                            )
                        pt = ptiles.tile([P, 2, 512], BF, tag="p")
                        if j >= 4 * i:
                            t = j - 4 * i
                            # cols < 128t are never computed nor read
                            nc.scalar.activation(
                                pt[:, :, P * t :], sp[:, :, P * t :],
                                Act.Exp, scale=0.125,
                            )
                            nc.gpsimd.tensor_mul(
                                pt[:, :, P * t : P * (t + 1)],
                                pt[:, :, P * t : P * (t + 1)],
                                cmask[:],
                            )
                        else:
                            nc.scalar.activation(pt[:], sp[:], Act.Exp, scale=0.125)
                        pts.append(pt)
                    op0 = psum_o.tile([P, 512], F32, tag="o")
                    op1 = psum_o.tile([P, 512], F32, tag="o")
                    ops = [op0, op1]
                    for j in range(jmax + 1):
                        # diagonal tile t only contributes to columns >= 128t
                        # (pt is zero below); j=0 is always the full-width
                        # start=True writer, so partial-width accumulates are
                        # safe for every element
                        c0 = P * (j - 4 * i) if j >= 4 * i else 0
                        for half in range(2):
                            h = 2 * hk + half
                            nc.tensor.matmul(
                                ops[half][0:65, c0:],
                                v_sb[:, j, h, 0:65],
                                pts[j][:, half, c0:],
                                start=(j == 0),
                                stop=(j == jmax),
                            )
                    # evict O^T (unnormalized) + l rows, then normalize.
                    # The l row is broadcast across partitions with a K=1 PE
                    # matmul (ones lhsT) — deliberately NO DMA and NO gpsimd
                    # in this chain: staging DMAs queue behind the bulk x/Y
                    # ring traffic (~7us each on HW), and partition-offset
                    # APs break the gpsimd/custom-DVE library ops on HW.
                    # reciprocal_approx_fast (~18 bits) is plenty for bf16.
                    lt = ltmp.tile([P, 2, 512], BF, tag="lt")
                    # l rows first (they gate the bcast->recip->mult chain):
                    # half h's l lands on partition 64+h so one K=2 matmul
                    # against the selector rows broadcasts both (rows 0-63
                    # <- l_h0, rows 64-127 <- l_h1), then one recip and one
                    # full-height multiply normalize the whole unit.
                    last_unit = i == NQ - 1 and hk == DH // P - 1
                    if last_unit:
                        # the final chain is pure latency: run the two l
                        # copies on different engines (ACT is out of EXPs)
                        nc.scalar.copy(lt[64:65, 0, :], ops[0][64:65, :])
                        nc.vector.tensor_copy(
                            lt[64:65, 1, :], ops[1][64:65, :]
                        )
                    else:
                        for half in range(2):
                            nc.vector.tensor_copy(
                                lt[64:65, half, :], ops[half][64:65, :]
                            )
                    if i == NQ - 1:
                        # tail units: ACT is about to run dry of EXPs, so
                        # split the evictions across engines for latency
                        nc.scalar.copy(
                            oT_sb[0:64, hk, ts(i, 512)], ops[0][0:64, :]
                        )
                    else:
                        nc.vector.tensor_copy(
                            oT_sb[0:64, hk, ts(i, 512)], ops[0][0:64, :]
                        )
                    nc.vector.tensor_copy(
                        oT_sb[64:128, hk, ts(i, 512)], ops[1][0:64, :]
                    )
                    # NOT the psum_p ring: FIFO slot rotation there would
                    # make later projection groups wait on this unit's recip
                    # chain, serializing the PE filler behind attention
                    lb = psum_o.tile([P, 512], F32, tag="o")
                    for half in range(2):
                        nc.tensor.matmul(
                            lb[:],
                            sel_t[64:65, half, :],
                            lt[64:65, half, :],
                            start=(half == 0),
                            stop=(half == 1),
                        )
                    rec = rbpool.tile([P, 512], F32, tag="rec")
                    nc.vector.reciprocal_approx_fast(rec[:], lb[:])
                    pending_mults.append(
                        (
                            oT_sb[:, hk, ts(i, 512)],
                            oT_sb[:, hk, ts(i, 512)],
                            rec[:],
                        )
                    )
                    if last_unit:
                        # DVFS warmers: anchored in the freed psum_o slots,
                        # these keep the PE busy across the final recip
                        # chain's dependency gap so the last Y groups run at
                        # full clock instead of restarting the 3us ramp.
                        # Results are never read; only 3 allocations follow
                        # in this ring, so nothing ever waits on them.
                        for _ in range(3):
                            pw = psum_o.tile([P, 512], F32, tag="o")
                            nc.tensor.matmul(
                                pw[:],
                                oT_sb[:, 0, ts(4 * i, P)],
                                wo_sb[:, 0, ts(0, 512)],
                                start=True,
                                stop=True,
                            )
                    # flush the previous unit's deferred normalizes now that
                    # their chains have had a full unit of slack
                    for args in prev_mults:
                        nc.vector.tensor_mul(*args)
                    if hk == 0 and i > 0:
                        # Y for t-range i-1 becomes eligible here (both head
                        # pairs normalized: prev_mults flushed above); its 4
                        # tiles are spread half-per-unit as steady PE filler
                        pending_yproj.extend(range(4 * (i - 1), 4 * i))
                    flush_half_yproj()

            # tail: flush the last deferred normalizes and final Y ranges
            for args in pending_mults:
                nc.vector.tensor_mul(*args)
            y_proj(pending_yproj, tail=True)
            y_proj(range(4 * (NQ - 1), 4 * NQ), tail=True)

    nc.compile()
    return nc


def _get_compiled():
    global _compiled
    if _compiled is None:
        _compiled = _build()
    return _compiled


def make_inputs(x, Wq, Wk, Wv, Wo):
    """Shard the full inputs into the 8 per-core input maps (host-side prep)."""
    import ml_dtypes

    bf16 = ml_dtypes.bfloat16
    x = np.asarray(x)
    in_maps = []
    for c in range(N_CORES):
        b, g = divmod(c, HPC)
        rows = slice(g * DH, (g + 1) * DH)
        in_maps.append(
            {
                "xT": np.ascontiguousarray(x[b].T).astype(bf16),
                "wqT": np.ascontiguousarray(np.asarray(Wq)[rows, :].T).astype(bf16),
                "wkT": np.ascontiguousarray(np.asarray(Wk)[rows, :].T).astype(bf16),
                "wvT": np.ascontiguousarray(np.asarray(Wv)[rows, :].T).astype(bf16),
                "woT": np.ascontiguousarray(np.asarray(Wo)[:, rows].T).astype(bf16),
            }
        )
    return in_maps


def assemble(results):
    """Sum the 4 tensor-parallel partials per batch into the full output."""
    y = np.zeros((B, T, C), dtype=np.float32)
    for c in range(N_CORES):
        b = c // HPC
        y[b] += np.asarray(results[c]["y"]).astype(np.float32)
    return y


def kernel(x, Wq, Wk, Wv, Wo):
    from concourse.bass_utils import run_bass_kernel_spmd

    nc = _get_compiled()
    in_maps = make_inputs(x, Wq, Wk, Wv, Wo)
    res = run_bass_kernel_spmd(nc, in_maps, list(range(N_CORES)))
    return assemble(res.results)
